# revision 22
# baseline (speedup 1.0000x reference)
"""Two-layer GCN (ClinicalGCN) on 8 Trainium2 NeuronCores.

Math (fold dinv[src] into x on host; defer dinv[dst] of layer 1 through
the relu using relu(a*x) = a*relu(x) for a>0, which holds since
self-loops make deg >= 1):
    h_hat[u]  = ((x[u]*dinv[u]) @ W1)            -> htab (full, per core)
    A1[v]     = sum_{e: dst=v} h_hat[src_e]      (segment sum, transposed)
    zero b1:  zr[v] = relu(A1[v]) @ W2           ws3_e = dinv^2[src]*dinv[dst]
    else:     zr[v] = relu(dinv[v]*A1[v]+b1)@W2  ws3_e = dinv[src]*dinv[dst]
    out[i]    = sum_{e: dst=i} ws3_e * zr[src_e] + b2

Device mapping:
  - Phase 1 is computed redundantly on every core (full x streamed in,
    full h_hat table written to the core's own DRAM).  This replaces a
    12.8MB AllGather (~250us) with ~110us of local DMA+PE.
  - Phase 2 dst-shards nodes (49 blocks of 128 per core).  Messages are
    fetched with grouped gpsimd.dma_gather (256B rows, int16 indices,
    table split in two halves), routed into PSUM with per-chunk
    selection matmuls (lhsT=msg, rhs=sel) that directly produce the
    TRANSPOSED aggregate [feat, node] - so relu+b1 is a partition-axis
    ACT op and @W2 needs no transpose.
  - Only the tiny zr table [tabn, 4] bf16 is AllGather'd, then expanded
    into 256B-strided rows so phase 3 can reuse the SAME gather indices
    and selection data as phase 2.
  - Output is accumulated transposed [4, pshard] and fixed up on host.
"""

import math

import ml_dtypes
import numpy as np

import concourse.bacc as bacc
import concourse.bass as bass
import concourse.mybir as mybir
import concourse.tile as tile
from concourse.bass_utils import run_bass_kernel_spmd

P = 128
N_CORES = 8
BF16 = ml_dtypes.bfloat16
ILV = 4        # phase-1 row interleave (1KB DMA descriptors)
GSZ = 4        # dst blocks per gather group


class Cfg:
    def __init__(self, n_nodes, n_in, n_hid, n_out, n_cores=N_CORES):
        assert n_nodes % n_cores == 0
        self.n = n_nodes
        self.nin = n_in
        self.nh = n_hid
        self.nc_out = n_out
        self.cores = n_cores
        self.shard = n_nodes // n_cores           # real nodes per core
        self.nblk = (self.shard + P - 1) // P     # dst blocks per core
        self.pshard = self.nblk * P               # padded nodes per core
        self.tabn = self.pshard * n_cores         # table rows (global)
        assert self.tabn % 2 == 0 and self.tabn % (ILV * P) == 0
        self.half = self.tabn // 2                # rows per table half
        assert self.half <= 32768, "int16 dma_gather index limit"
        self.kin = n_in // P                      # k chunks for x @ W1
        self.ngrp1 = self.tabn // (ILV * P)       # phase-1 write groups
        # phase-2/3 gather groups over the local dst blocks
        self.groups = [list(range(g, min(g + GSZ, self.nblk)))
                       for g in range(0, self.nblk, GSZ)]


FULL = Cfg(50000, 256, 128, 4)


# ---------------------------------------------------------------- host prep
def _wrap_idx(idx):
    """int16 idx array [n*128] -> dma_gather layout [128, n*8]."""
    n = idx.shape[0] // P
    w = idx.reshape(n * 8, 16).T            # [16, n*8]
    return np.tile(w, (8, 1))               # [128, n*8]


def host_prep(cfg: Cfg, x, edge_index, W1, b1, W2, b2):
    """Build per-core input arrays. Pure numpy."""
    n = cfg.n
    src = np.concatenate([edge_index[0], np.arange(n, dtype=np.int64)])
    dst = np.concatenate([edge_index[1], np.arange(n, dtype=np.int64)])
    deg = np.bincount(dst, minlength=n).astype(np.float32)
    dinv = np.where(deg > 0, 1.0 / np.sqrt(deg), 0.0).astype(np.float32)
    zero_bias = not (np.any(b1) or np.any(b2))

    # padded-table row index for each global node id
    trow = ((src // cfg.shard) * cfg.pshard + src % cfg.shard).astype(np.int64)

    # phase-1 lhsT: x*dinv padded+transposed, columns permuted so matmul m
    # of write-group g produces table rows {ILV*P*g + ILV*q + m}.
    xd = (x * dinv[:, None]).astype(np.float32)
    X0 = np.zeros((cfg.nin, cfg.tabn), dtype=BF16)
    cols = ((np.arange(n) // cfg.shard) * cfg.pshard + np.arange(n) % cfg.shard)
    X0[:, cols] = xd.T.astype(BF16)
    xdT = (X0.reshape(cfg.nin, cfg.ngrp1, P, ILV)
           .transpose(0, 1, 3, 2).reshape(cfg.nin, cfg.tabn))

    # order edges by destination, group by (core, block, half)
    order = np.argsort(dst, kind="stable")
    dst_s = dst[order]
    trow_s = trow[order]
    src_s = src[order]
    ldl_s = dst_s % cfg.shard
    lslot_s = (ldl_s % P).astype(np.float32)
    half_s = (trow_s >= cfg.half).astype(np.int64)
    blk_s = (dst_s // cfg.shard) * cfg.nblk + ldl_s // P
    ws2_s = dinv[dst_s]

    nblk_total = cfg.cores * cfg.nblk
    cnt = np.zeros((nblk_total, 2), dtype=np.int64)
    np.add.at(cnt, (blk_s, half_s), 1)
    cnt3 = cnt.reshape(cfg.cores, cfg.nblk, 2)
    # chunk counts per (block, half): max across cores (shared SPMD program)
    KH = [np.maximum(1, np.ceil(cnt3[:, :, h].max(axis=0) / P)).astype(int)
          for h in range(2)]

    key = blk_s * 2 + half_s
    order2 = np.argsort(key, kind="stable")
    trow2 = trow_s[order2]
    lslot2 = lslot_s[order2]
    ws2_2 = ws2_s[order2]
    key2 = key[order2]
    starts = np.searchsorted(key2, np.arange(nblk_total * 2 + 1))

    KA, KB = KH
    Ksum = KA + KB
    Ktot = int(Ksum.sum())
    # column offset of each block's chunks in the resident ldst/ws tiles
    coff = np.concatenate([[0], np.cumsum(Ksum)]).astype(int)
    # per-node scale folded into the zr table (dinv^2 when the relu trick
    # absorbs both dinv factors, else dinv), and the final dinv[dst] scale
    dtab = dinv * dinv if zero_bias else dinv
    per_core = []
    for c in range(cfg.cores):
        ldst = np.full((P, Ktot), -1.0, dtype=BF16)
        ws2 = np.zeros((P, Ktot), dtype=np.float32)
        gidx_core = []
        for grp in cfg.groups:
            for h in range(2):
                Kh = KA if h == 0 else KB
                parts = []
                for b in grp:
                    g = (c * cfg.nblk + b) * 2 + h
                    lo, hi = starts[g], starts[g + 1]
                    cnt_e = hi - lo
                    idx = np.zeros(Kh[b] * P, dtype=np.int16)
                    idx[:cnt_e] = trow2[lo:hi] - h * cfg.half
                    parts.append(_wrap_idx(idx))
                    t = np.arange(cnt_e)
                    j0 = coff[b] + (0 if h == 0 else KA[b])
                    ldst[t % P, j0 + t // P] = lslot2[lo:hi].astype(BF16)
                    ws2[t % P, j0 + t // P] = ws2_2[lo:hi]
                gidx_core.append(np.concatenate(parts, axis=1))
        gidx = np.concatenate(gidx_core, axis=1)  # [128, Ktot*8]
        nloc = np.arange(cfg.pshard)
        gl = np.minimum(c * cfg.shard + nloc, cfg.n - 1)
        dvs = dtab[gl].reshape(cfg.nblk, P).T.astype(np.float32).copy()
        dvo = np.broadcast_to(dinv[gl].astype(BF16),
                              (cfg.nc_out, cfg.pshard)).copy()
        pc = {"ldst": ldst, "gidx": gidx, "dvs": dvs, "dvo": dvo}
        if not zero_bias:
            pc["ws2"] = ws2
        per_core.append(pc)

    iota = np.broadcast_to(np.arange(P, dtype=np.float32).astype(BF16),
                           (P, P)).copy()
    shared = {
        "xdT": xdT,
        "W1": W1.astype(BF16),
        "W2": W2.astype(BF16),
        "iota": iota,
    }
    if not zero_bias:
        shared["b1c"] = b1.astype(np.float32).reshape(cfg.nh, 1).copy()
        shared["b2c"] = b2.astype(np.float32).reshape(cfg.nc_out, 1).copy()
    in_maps = [{**shared, **pc} for pc in per_core]
    return in_maps, KH, zero_bias


# --------------------------------------------------------------- bass build
def build_nc(cfg: Cfg, KH, zero_bias):
    f32 = mybir.dt.float32
    bf16 = mybir.dt.bfloat16
    i16 = mybir.dt.int16
    KA, KB = [np.asarray(k, dtype=np.int64) for k in KH]
    Ksum = KA + KB
    Ktot = int(Ksum.sum())
    coff = np.concatenate([[0], np.cumsum(Ksum)]).astype(int)
    # idx column offsets per (grp, half) in the resident gidx tile
    gio = [0]
    gmeta = []   # per (grp, half): (blocks, Ksub, idx_off)
    for grp in cfg.groups:
        for h in range(2):
            Kh = KA if h == 0 else KB
            nidx = int(sum(Kh[b] for b in grp)) * P
            gmeta.append((grp, h, gio[-1]))
            gio.append(gio[-1] + nidx // 16)

    nc = bacc.Bacc("TRN2", target_bir_lowering=False, debug=False,
                   num_devices=cfg.cores, num_swdge_queues=4)

    xdT = nc.dram_tensor("xdT", [cfg.nin, cfg.tabn], bf16,
                         kind="ExternalInput")
    W1 = nc.dram_tensor("W1", [cfg.nin, cfg.nh], bf16, kind="ExternalInput")
    W2 = nc.dram_tensor("W2", [cfg.nh, cfg.nc_out], bf16,
                        kind="ExternalInput")
    iota = nc.dram_tensor("iota", [P, P], bf16, kind="ExternalInput")
    gidx = nc.dram_tensor("gidx", [P, gio[-1]], i16, kind="ExternalInput")
    ldst = nc.dram_tensor("ldst", [P, Ktot], bf16, kind="ExternalInput")
    dvs = nc.dram_tensor("dvs", [P, cfg.nblk], f32, kind="ExternalInput")
    dvo = nc.dram_tensor("dvo", [cfg.nc_out, cfg.pshard], bf16,
                         kind="ExternalInput")
    if not zero_bias:
        ws2 = nc.dram_tensor("ws2", [P, Ktot], f32, kind="ExternalInput")
        b1c = nc.dram_tensor("b1c", [cfg.nh, 1], f32, kind="ExternalInput")
        b2c = nc.dram_tensor("b2c", [cfg.nc_out, 1], f32,
                             kind="ExternalInput")
    out = nc.dram_tensor("out", [cfg.nc_out, cfg.pshard], f32,
                         kind="ExternalOutput")

    relu = mybir.ActivationFunctionType.Relu

    with tile.TileContext(nc) as tc:
        with (
            tc.tile_pool(name="const", bufs=1) as cpool,
            tc.tile_pool(name="x", bufs=3) as xpool,
            tc.tile_pool(name="h", bufs=3) as hpool,
            tc.tile_pool(name="msg", bufs=3) as mpool,
            tc.tile_pool(name="sel", bufs=2) as spool,
            tc.tile_pool(name="ps", bufs=2, space="PSUM") as pspool,
            tc.tile_pool(name="dram", bufs=1, space="DRAM") as dram,
        ):
            # ---- resident constants / metadata
            w1t = cpool.tile([P, cfg.kin * cfg.nh], bf16, tag="w1")
            nc.sync.dma_start(
                out=w1t[:].rearrange("p (a d) -> p a d", a=cfg.kin),
                in_=W1[:].rearrange("(a p) d -> p a d", p=P))
            w2t = cpool.tile([cfg.nh, cfg.nc_out], bf16, tag="w2")
            nc.sync.dma_start(out=w2t[:], in_=W2[:])
            iot = cpool.tile([P, P], bf16, tag="iota")
            nc.sync.dma_start(out=iot[:], in_=iota[:])
            git = cpool.tile([P, gio[-1]], i16, tag="gidx")
            nc.sync.dma_start(out=git[:], in_=gidx[:])
            ldt = cpool.tile([P, Ktot], bf16, tag="ldst")
            nc.sync.dma_start(out=ldt[:], in_=ldst[:])
            dvst = cpool.tile([P, cfg.nblk], f32, tag="dvs")
            nc.sync.dma_start(out=dvst[:], in_=dvs[:])
            dvot = cpool.tile([cfg.nc_out, cfg.pshard], bf16, tag="dvo")
            nc.sync.dma_start(out=dvot[:], in_=dvo[:])
            if not zero_bias:
                ws2t = cpool.tile([P, Ktot], f32, tag="ws2")
                nc.sync.dma_start(out=ws2t[:], in_=ws2[:])
                b1t = cpool.tile([cfg.nh, 1], f32, tag="b1")
                nc.sync.dma_start(out=b1t[:], in_=b1c[:])
                b2t = cpool.tile([cfg.nc_out, 1], f32, tag="b2")
                nc.sync.dma_start(out=b2t[:], in_=b2c[:])

            htab = dram.tile([cfg.tabn, cfg.nh], bf16)
            zrsh = dram.tile([cfg.pshard, cfg.nc_out], bf16)
            zrtab = dram.tile([cfg.tabn, cfg.nc_out], bf16,
                              addr_space="Shared")

            # ---------------- phase 1: full h_hat table, written interleaved
            for g in range(cfg.ngrp1):
                xt = xpool.tile([P, cfg.kin * ILV * P], bf16, tag="xt")
                nc.sync.dma_start(
                    out=xt[:].rearrange("p (a d) -> p a d", a=cfg.kin),
                    in_=xdT[:, g * ILV * P:(g + 1) * ILV * P]
                    .rearrange("(a p) d -> p a d", p=P))
                ot = hpool.tile([P, ILV * cfg.nh], bf16, tag="p1o")
                for m in range(ILV):
                    ps = pspool.tile([P, cfg.nh], f32, tag="ps_h",
                                     bufs=4)
                    for kk in range(cfg.kin):
                        nc.tensor.matmul(
                            out=ps[:],
                            lhsT=xt[:, (kk * ILV + m) * P:
                                    (kk * ILV + m + 1) * P],
                            rhs=w1t[:, kk * cfg.nh:(kk + 1) * cfg.nh],
                            start=(kk == 0), stop=(kk == cfg.kin - 1))
                    dst_sl = ot[:, m * cfg.nh:(m + 1) * cfg.nh]
                    if m % 2 == 0:
                        nc.scalar.copy(out=dst_sl, in_=ps[:])
                    else:
                        nc.vector.tensor_copy(out=dst_sl, in_=ps[:])
                nc.scalar.dma_start(
                    out=htab[g * ILV * P:(g + 1) * ILV * P, :]
                    .rearrange("(q i) f -> q (i f)", i=ILV),
                    in_=ot[:])

            # helper: grouped gather of 256B rows for blocks of group gi
            Kgmax = max(int(sum(Ksum[b] for b in grp)) for grp in cfg.groups)

            def gather_group(gi, table):
                # stripe each half's chunks across the 4 SWDGE queues so
                # their DMA rings drain in parallel
                grp = cfg.groups[gi]
                msg = mpool.tile([P, Kgmax * cfg.nh], bf16, tag="msg")
                j0 = 0
                for h in range(2):
                    off = gmeta[2 * gi + h][2]
                    Kh = KA if h == 0 else KB
                    nsub = int(sum(Kh[b] for b in grp))
                    # 4 even stripes on queues 0-3 (period 4 keeps the
                    # 8-lane DMASW sem rotation queue-consistent); each
                    # ring drains at ~8-9ns/descriptor, in parallel.
                    splits = np.linspace(0, nsub, 5).astype(int)
                    for q, qn in enumerate((0, 1, 2, 3)):
                        ks, ke = int(splits[q]), int(splits[q + 1])
                        nq = max(ke - ks, 0)
                        if nq == 0:
                            ks, ke, nq = 0, 1, 1  # keep sem pattern aligned
                        nc.gpsimd.dma_gather(
                            out_ap=msg[:, (j0 + ks) * cfg.nh:
                                       (j0 + ke) * cfg.nh]
                            .rearrange("p (k f) -> p k f", k=nq),
                            in_ap=table[h * cfg.half:(h + 1) * cfg.half, :],
                            idxs_ap=git[:, off + ks * 8:off + ke * 8],
                            num_idxs=nq * P,
                            num_idxs_reg=nq * P,
                            elem_size=cfg.nh,
                            single_packet=False,
                            queue_num=qn)
                    j0 += nsub
                # column offset of block b's half-h chunks inside msg
                moff = {}
                pos = 0
                for h in range(2):
                    Kh = KA if h == 0 else KB
                    for b in grp:
                        moff[(b, h)] = pos
                        pos += int(Kh[b])
                return msg, moff

            # staging tile for the post-AllGather expansion; zeroed once so
            # pad lanes of the phase-3 table are 0.0 rather than garbage
            qa_e = next(q for q in (28, 16, 14, 8, 7, 4, 2, 1)
                        if (cfg.tabn // P) % q == 0)
            zpad = cpool.tile([P, 2 * qa_e * cfg.nh], bf16, tag="zpad")
            nc.scalar.memzero(zpad[:])

            # ---------------- phase 2: zr table (transposed aggregation)
            zrall = hpool.tile([P, cfg.nblk * cfg.nc_out], bf16, tag="zrall")
            for gi in range(len(cfg.groups)):
                msg, moff = gather_group(gi, htab)
                for b in cfg.groups[gi]:
                    K_b = int(Ksum[b])
                    sel = spool.tile([P, K_b * P], bf16, tag="sel")
                    nc.vector.tensor_tensor(
                        out=sel[:].rearrange("p (k f) -> p k f", k=K_b),
                        in0=ldt[:, coff[b]:coff[b + 1], None]
                        .to_broadcast([P, K_b, P]),
                        in1=iot[:, None, :].to_broadcast([P, K_b, P]),
                        op=mybir.AluOpType.is_equal)
                    ps = pspool.tile([P, cfg.nh], f32, tag="ps_agg")
                    nch = 0
                    for h in range(2):
                        Kh = int((KA if h == 0 else KB)[b])
                        m0 = moff[(b, h)]
                        c0 = 0 if h == 0 else int(KA[b])
                        for k in range(Kh):
                            if not zero_bias:
                                nc.vector.tensor_scalar_mul(
                                    out=msg[:, (m0 + k) * cfg.nh:
                                            (m0 + k + 1) * cfg.nh],
                                    in0=msg[:, (m0 + k) * cfg.nh:
                                            (m0 + k + 1) * cfg.nh],
                                    scalar1=ws2t[:, coff[b] + c0 + k:
                                                 coff[b] + c0 + k + 1])
                            nc.tensor.matmul(
                                out=ps[:],
                                lhsT=msg[:, (m0 + k) * cfg.nh:
                                         (m0 + k + 1) * cfg.nh],
                                rhs=sel[:, (c0 + k) * P:(c0 + k + 1) * P],
                                start=(nch == 0), stop=(nch == K_b - 1))
                            nch += 1
                    rt = hpool.tile([P, cfg.nh], bf16, tag="rt")
                    nc.scalar.activation(
                        out=rt[:], in_=ps[:], func=relu,
                        bias=0.0 if zero_bias else b1t[:, 0:1])
                    ps2 = pspool.tile([P, cfg.nc_out], f32, tag="ps_sm")
                    nc.tensor.matmul(out=ps2[:], lhsT=rt[:], rhs=w2t[:],
                                     start=True, stop=True)
                    nc.scalar.mul(
                        out=zrall[:, b * cfg.nc_out:(b + 1) * cfg.nc_out],
                        in_=ps2[:], mul=dvst[:, b:b + 1])
            nc.sync.dma_start(
                out=zrsh[:].rearrange("(b p) c -> p b c", p=P),
                in_=zrall[:].rearrange("p (b c) -> p b c", b=cfg.nblk))

            nc.gpsimd.collective_compute(
                "AllGather", mybir.AluOpType.bypass,
                replica_groups=[list(range(cfg.cores))],
                ins=[zrsh.opt()], outs=[zrtab.opt()])

            # expand compact zr rows into 256B-strided gatherable rows.
            # A direct strided DRAM->DRAM copy costs ~178us (8B descriptors);
            # instead stage through a pre-zeroed SBUF tile so both DMAs use
            # large per-partition-contiguous descriptors.
            qa = qa_e
            Q = qa * P
            NEXP = cfg.tabn // Q
            for c in range(NEXP):
                zin = hpool.tile([P, qa * cfg.nc_out], bf16, tag="zin")
                nc.scalar.dma_start(
                    out=zin[:].rearrange("p (a c) -> p a c", a=qa),
                    in_=zrtab[c * Q:(c + 1) * Q, :]
                    .rearrange("(p a) c -> p a c", p=P))
                zp = zpad[:, (c % 2) * qa * cfg.nh:
                          (c % 2 + 1) * qa * cfg.nh]
                nc.vector.tensor_copy(
                    out=zp.rearrange("p (a f) -> p a f", a=qa)
                    [:, :, 0:cfg.nc_out],
                    in_=zin[:].rearrange("p (a c) -> p a c", a=qa))
                nc.scalar.dma_start(
                    out=htab[c * Q:(c + 1) * Q, :]
                    .rearrange("(p a) f -> p (a f)", p=P),
                    in_=zp)

            # ---------------- phase 3: out.T = sum sel-routed ws3*zr[src]
            oall = hpool.tile([cfg.nc_out, cfg.nblk * P], f32, tag="oall")
            for gi in range(len(cfg.groups)):
                msg, moff = gather_group(gi, htab)
                for b in cfg.groups[gi]:
                    K_b = int(Ksum[b])
                    sel = spool.tile([P, K_b * P], bf16, tag="sel")
                    nc.vector.tensor_tensor(
                        out=sel[:].rearrange("p (k f) -> p k f", k=K_b),
                        in0=ldt[:, coff[b]:coff[b + 1], None]
                        .to_broadcast([P, K_b, P]),
                        in1=iot[:, None, :].to_broadcast([P, K_b, P]),
                        op=mybir.AluOpType.is_equal)
                    ps = pspool.tile([cfg.nc_out, P], f32, tag="ps_sm")
                    nch = 0
                    for h in range(2):
                        Kh = int((KA if h == 0 else KB)[b])
                        m0 = moff[(b, h)]
                        c0 = 0 if h == 0 else int(KA[b])
                        for k in range(Kh):
                            nc.tensor.matmul(
                                out=ps[:],
                                lhsT=msg[:, (m0 + k) * P:
                                         (m0 + k) * P + cfg.nc_out],
                                rhs=sel[:, (c0 + k) * P:(c0 + k + 1) * P],
                                start=(nch == 0), stop=(nch == K_b - 1))
                            nch += 1
                    nc.scalar.copy(
                        out=oall[:, b * P:(b + 1) * P], in_=ps[:])
            nc.vector.tensor_tensor(out=oall[:], in0=oall[:], in1=dvot[:],
                                    op=mybir.AluOpType.mult)
            if not zero_bias:
                nc.vector.tensor_scalar_add(out=oall[:], in0=oall[:],
                                            scalar1=b2t[:, 0:1])
            nc.sync.dma_start(out=out[:, :], in_=oall[:])

    nc.compile()
    return nc


# ------------------------------------------------------------------ driver
def kernel(x, edge_index, W1, b1, W2, b2):
    cfg = FULL
    assert x.shape == (cfg.n, cfg.nin)
    in_maps, KH, zero_bias = host_prep(
        cfg, np.asarray(x), np.asarray(edge_index), np.asarray(W1),
        np.asarray(b1), np.asarray(W2), np.asarray(b2))
    nc = build_nc(cfg, KH, zero_bias)
    res = run_bass_kernel_spmd(nc, in_maps, core_ids=list(range(cfg.cores)))
    parts = [res.results[c]["out"][:, :cfg.shard].T
             for c in range(cfg.cores)]
    return np.concatenate(parts, axis=0).astype(np.float32)


# revision 26
# speedup vs baseline: 1.0223x; 1.0223x over previous
"""Two-layer GCN (ClinicalGCN) on 8 Trainium2 NeuronCores.

Math (fold dinv[src] into x on host; defer dinv[dst] of layer 1 through
the relu using relu(a*x) = a*relu(x) for a>0, which holds since
self-loops make deg >= 1):
    h_hat[u]  = ((x[u]*dinv[u]) @ W1)            -> htab (full, per core)
    A1[v]     = sum_{e: dst=v} h_hat[src_e]      (segment sum, transposed)
    zero b1:  zr[v] = relu(A1[v]) @ W2           ws3_e = dinv^2[src]*dinv[dst]
    else:     zr[v] = relu(dinv[v]*A1[v]+b1)@W2  ws3_e = dinv[src]*dinv[dst]
    out[i]    = sum_{e: dst=i} ws3_e * zr[src_e] + b2

Device mapping:
  - Phase 1 is computed redundantly on every core (full x streamed in,
    full h_hat table written to the core's own DRAM).  This replaces a
    12.8MB AllGather (~250us) with ~110us of local DMA+PE.
  - Phase 2 dst-shards nodes (49 blocks of 128 per core).  Messages are
    fetched with grouped gpsimd.dma_gather (256B rows, int16 indices,
    table split in two halves), routed into PSUM with per-chunk
    selection matmuls (lhsT=msg, rhs=sel) that directly produce the
    TRANSPOSED aggregate [feat, node] - so relu+b1 is a partition-axis
    ACT op and @W2 needs no transpose.
  - Only the tiny zr table [tabn, 4] bf16 is AllGather'd, then expanded
    into 256B-strided rows so phase 3 can reuse the SAME gather indices
    and selection data as phase 2.
  - Output is accumulated transposed [4, pshard] and fixed up on host.
"""

import math

import ml_dtypes
import numpy as np

import concourse.bacc as bacc
import concourse.bass as bass
import concourse.mybir as mybir
import concourse.tile as tile
from concourse.bass_utils import run_bass_kernel_spmd

P = 128
N_CORES = 8
BF16 = ml_dtypes.bfloat16
ILV = 4        # phase-1 row interleave (1KB DMA descriptors)
GSZ = 4        # dst blocks per gather group


class Cfg:
    def __init__(self, n_nodes, n_in, n_hid, n_out, n_cores=N_CORES):
        assert n_nodes % n_cores == 0
        self.n = n_nodes
        self.nin = n_in
        self.nh = n_hid
        self.nc_out = n_out
        self.cores = n_cores
        self.shard = n_nodes // n_cores           # real nodes per core
        self.nblk = (self.shard + P - 1) // P     # dst blocks per core
        self.pshard = self.nblk * P               # padded nodes per core
        self.tabn = self.pshard * n_cores         # table rows (global)
        assert self.tabn % 2 == 0 and self.tabn % (ILV * P) == 0
        self.half = self.tabn // 2                # rows per table half
        assert self.half <= 32768, "int16 dma_gather index limit"
        self.kin = n_in // P                      # k chunks for x @ W1
        self.ngrp1 = self.tabn // (ILV * P)       # phase-1 write groups
        # phase-2/3 gather groups over the local dst blocks
        self.groups = [list(range(g, min(g + GSZ, self.nblk)))
                       for g in range(0, self.nblk, GSZ)]


FULL = Cfg(50000, 256, 128, 4)


# ---------------------------------------------------------------- host prep
def _wrap_idx(idx):
    """int16 idx array [n*128] -> dma_gather layout [128, n*8]."""
    n = idx.shape[0] // P
    w = idx.reshape(n * 8, 16).T            # [16, n*8]
    return np.tile(w, (8, 1))               # [128, n*8]


def host_prep(cfg: Cfg, x, edge_index, W1, b1, W2, b2):
    """Build per-core input arrays. Pure numpy."""
    n = cfg.n
    src = np.concatenate([edge_index[0], np.arange(n, dtype=np.int64)])
    dst = np.concatenate([edge_index[1], np.arange(n, dtype=np.int64)])
    deg = np.bincount(dst, minlength=n).astype(np.float32)
    dinv = np.where(deg > 0, 1.0 / np.sqrt(deg), 0.0).astype(np.float32)
    zero_bias = not (np.any(b1) or np.any(b2))

    # original padded-table row; only used for the half split, which is
    # invariant under the within-core balancing permutation below
    trow0 = ((src // cfg.shard) * cfg.pshard
             + src % cfg.shard).astype(np.int64)
    half_bit = (trow0 >= cfg.half).astype(np.int64)

    # --- balance dst nodes over blocks (2D LPT on per-half degree) so the
    # per-(block, half) chunk counts quantize to the minimum K everywhere.
    degA = np.zeros(n, np.int64)
    degB = np.zeros(n, np.int64)
    np.add.at(degA, dst[half_bit == 0], 1)
    np.add.at(degB, dst[half_bit == 1], 1)
    ppos = np.zeros(n, np.int64)        # global node -> padded position
    for c in range(cfg.cores):
        nodes = np.arange(c * cfg.shard, (c + 1) * cfg.shard)
        dA = degA[nodes]
        dB = degB[nodes]
        order = np.argsort(-(dA + dB), kind="stable")
        sumA = np.zeros(cfg.nblk, np.int64)
        sumB = np.zeros(cfg.nblk, np.int64)
        used = np.zeros(cfg.nblk, np.int64)
        BIG = 1 << 50
        for v in order:
            score = np.maximum(sumA + dA[v], sumB + dB[v]) + used * 2
            score[used >= P] = BIG
            b = int(score.argmin())
            ppos[nodes[v]] = c * cfg.pshard + b * P + used[b]
            sumA[b] += dA[v]
            sumB[b] += dB[v]
            used[b] += 1

    # phase-1 lhsT: x*dinv padded+transposed; the table row of node v is
    # its balanced position ppos[v] (so phases 2 and 3 share one layout),
    # and columns are further permuted so matmul m of write-group g
    # produces table rows {ILV*P*g + ILV*q + m}.
    trow = ppos[src]
    xd = (x * dinv[:, None]).astype(np.float32)
    X0 = np.zeros((cfg.nin, cfg.tabn), dtype=BF16)
    X0[:, ppos[np.arange(n)]] = xd.T.astype(BF16)
    xdT = (X0.reshape(cfg.nin, cfg.ngrp1, P, ILV)
           .transpose(0, 1, 3, 2).reshape(cfg.nin, cfg.tabn))

    # order edges by destination, group by (core, block, half)
    order = np.argsort(dst, kind="stable")
    dst_s = dst[order]
    trow_s = trow[order]
    src_s = src[order]
    pp_s = ppos[dst_s]
    lslot_s = (pp_s % P).astype(np.float32)
    half_s = (trow_s >= cfg.half).astype(np.int64)
    blk_s = pp_s // P
    ws2_s = dinv[dst_s]

    nblk_total = cfg.cores * cfg.nblk
    cnt = np.zeros((nblk_total, 2), dtype=np.int64)
    np.add.at(cnt, (blk_s, half_s), 1)
    cnt3 = cnt.reshape(cfg.cores, cfg.nblk, 2)
    # chunk counts per (block, half): max across cores (shared SPMD program)
    KH = [np.maximum(1, np.ceil(cnt3[:, :, h].max(axis=0) / P)).astype(int)
          for h in range(2)]

    key = blk_s * 2 + half_s
    order2 = np.argsort(key, kind="stable")
    trow2 = trow_s[order2]
    lslot2 = lslot_s[order2]
    ws2_2 = ws2_s[order2]
    key2 = key[order2]
    starts = np.searchsorted(key2, np.arange(nblk_total * 2 + 1))

    KA, KB = KH
    Ksum = KA + KB
    Ktot = int(Ksum.sum())
    # column offset of each block's chunks in the resident ldst/ws tiles
    coff = np.concatenate([[0], np.cumsum(Ksum)]).astype(int)
    # per-node scale folded into the zr table (dinv^2 when the relu trick
    # absorbs both dinv factors, else dinv), and the final dinv[dst] scale
    dtab = dinv * dinv if zero_bias else dinv
    per_core = []
    for c in range(cfg.cores):
        ldst = np.full((P, Ktot), -1.0, dtype=BF16)
        ws2 = np.zeros((P, Ktot), dtype=np.float32)
        gidx_core = []
        for grp in cfg.groups:
            for h in range(2):
                Kh = KA if h == 0 else KB
                parts = []
                for b in grp:
                    g = (c * cfg.nblk + b) * 2 + h
                    lo, hi = starts[g], starts[g + 1]
                    cnt_e = hi - lo
                    idx = np.zeros(Kh[b] * P, dtype=np.int16)
                    idx[:cnt_e] = trow2[lo:hi] - h * cfg.half
                    parts.append(_wrap_idx(idx))
                    t = np.arange(cnt_e)
                    j0 = coff[b] + (0 if h == 0 else KA[b])
                    ldst[t % P, j0 + t // P] = lslot2[lo:hi].astype(BF16)
                    ws2[t % P, j0 + t // P] = ws2_2[lo:hi]
                gidx_core.append(np.concatenate(parts, axis=1))
        gidx = np.concatenate(gidx_core, axis=1)  # [128, Ktot*8]
        nodes = np.arange(c * cfg.shard, (c + 1) * cfg.shard)
        lpos = ppos[nodes] - c * cfg.pshard
        dvsf = np.zeros(cfg.pshard, np.float32)
        dvsf[lpos] = dtab[nodes]
        dvs = dvsf.reshape(cfg.nblk, P).T.copy()
        dvof = np.zeros(cfg.pshard, np.float32)
        dvof[lpos] = dinv[nodes]
        dvo = np.broadcast_to(dvof.astype(BF16),
                              (cfg.nc_out, cfg.pshard)).copy()
        pc = {"ldst": ldst, "gidx": gidx, "dvs": dvs, "dvo": dvo}
        if not zero_bias:
            pc["ws2"] = ws2
        per_core.append(pc)

    iota = np.broadcast_to(np.arange(P, dtype=np.float32).astype(BF16),
                           (P, P)).copy()
    shared = {
        "xdT": xdT,
        "W1": W1.astype(BF16),
        "W2": W2.astype(BF16),
        "iota": iota,
    }
    if not zero_bias:
        shared["b1c"] = b1.astype(np.float32).reshape(cfg.nh, 1).copy()
        shared["b2c"] = b2.astype(np.float32).reshape(cfg.nc_out, 1).copy()
    in_maps = [{**shared, **pc} for pc in per_core]
    return in_maps, KH, zero_bias, ppos


# --------------------------------------------------------------- bass build
def build_nc(cfg: Cfg, KH, zero_bias):
    f32 = mybir.dt.float32
    bf16 = mybir.dt.bfloat16
    i16 = mybir.dt.int16
    KA, KB = [np.asarray(k, dtype=np.int64) for k in KH]
    Ksum = KA + KB
    Ktot = int(Ksum.sum())
    coff = np.concatenate([[0], np.cumsum(Ksum)]).astype(int)
    # idx column offsets per (grp, half) in the resident gidx tile
    gio = [0]
    gmeta = []   # per (grp, half): (blocks, Ksub, idx_off)
    for grp in cfg.groups:
        for h in range(2):
            Kh = KA if h == 0 else KB
            nidx = int(sum(Kh[b] for b in grp)) * P
            gmeta.append((grp, h, gio[-1]))
            gio.append(gio[-1] + nidx // 16)

    nc = bacc.Bacc("TRN2", target_bir_lowering=False, debug=False,
                   num_devices=cfg.cores, num_swdge_queues=4)

    xdT = nc.dram_tensor("xdT", [cfg.nin, cfg.tabn], bf16,
                         kind="ExternalInput")
    W1 = nc.dram_tensor("W1", [cfg.nin, cfg.nh], bf16, kind="ExternalInput")
    W2 = nc.dram_tensor("W2", [cfg.nh, cfg.nc_out], bf16,
                        kind="ExternalInput")
    iota = nc.dram_tensor("iota", [P, P], bf16, kind="ExternalInput")
    gidx = nc.dram_tensor("gidx", [P, gio[-1]], i16, kind="ExternalInput")
    ldst = nc.dram_tensor("ldst", [P, Ktot], bf16, kind="ExternalInput")
    dvs = nc.dram_tensor("dvs", [P, cfg.nblk], f32, kind="ExternalInput")
    dvo = nc.dram_tensor("dvo", [cfg.nc_out, cfg.pshard], bf16,
                         kind="ExternalInput")
    if not zero_bias:
        ws2 = nc.dram_tensor("ws2", [P, Ktot], f32, kind="ExternalInput")
        b1c = nc.dram_tensor("b1c", [cfg.nh, 1], f32, kind="ExternalInput")
        b2c = nc.dram_tensor("b2c", [cfg.nc_out, 1], f32,
                             kind="ExternalInput")
    out = nc.dram_tensor("out", [cfg.nc_out, cfg.pshard], f32,
                         kind="ExternalOutput")

    relu = mybir.ActivationFunctionType.Relu

    with tile.TileContext(nc) as tc:
        with (
            tc.tile_pool(name="const", bufs=1) as cpool,
            tc.tile_pool(name="x", bufs=3) as xpool,
            tc.tile_pool(name="h", bufs=3) as hpool,
            tc.tile_pool(name="msg", bufs=3) as mpool,
            tc.tile_pool(name="sel", bufs=2) as spool,
            tc.tile_pool(name="ps", bufs=2, space="PSUM") as pspool,
            tc.tile_pool(name="dram", bufs=1, space="DRAM") as dram,
        ):
            # ---- resident constants / metadata
            w1t = cpool.tile([P, cfg.kin * cfg.nh], bf16, tag="w1")
            nc.sync.dma_start(
                out=w1t[:].rearrange("p (a d) -> p a d", a=cfg.kin),
                in_=W1[:].rearrange("(a p) d -> p a d", p=P))
            w2t = cpool.tile([cfg.nh, cfg.nc_out], bf16, tag="w2")
            nc.sync.dma_start(out=w2t[:], in_=W2[:])
            iot = cpool.tile([P, P], bf16, tag="iota")
            nc.sync.dma_start(out=iot[:], in_=iota[:])
            git = cpool.tile([P, gio[-1]], i16, tag="gidx")
            nc.sync.dma_start(out=git[:], in_=gidx[:])
            ldt = cpool.tile([P, Ktot], bf16, tag="ldst")
            nc.sync.dma_start(out=ldt[:], in_=ldst[:])
            dvst = cpool.tile([P, cfg.nblk], f32, tag="dvs")
            nc.sync.dma_start(out=dvst[:], in_=dvs[:])
            dvot = cpool.tile([cfg.nc_out, cfg.pshard], bf16, tag="dvo")
            nc.sync.dma_start(out=dvot[:], in_=dvo[:])
            if not zero_bias:
                ws2t = cpool.tile([P, Ktot], f32, tag="ws2")
                nc.sync.dma_start(out=ws2t[:], in_=ws2[:])
                b1t = cpool.tile([cfg.nh, 1], f32, tag="b1")
                nc.sync.dma_start(out=b1t[:], in_=b1c[:])
                b2t = cpool.tile([cfg.nc_out, 1], f32, tag="b2")
                nc.sync.dma_start(out=b2t[:], in_=b2c[:])

            htab = dram.tile([cfg.tabn, cfg.nh], bf16)
            zrsh = dram.tile([cfg.pshard, cfg.nc_out], bf16)
            zrtab = dram.tile([cfg.tabn, cfg.nc_out], bf16,
                              addr_space="Shared")

            # ---------------- phase 1: full h_hat table, written interleaved
            for g in range(cfg.ngrp1):
                xt = xpool.tile([P, cfg.kin * ILV * P], bf16, tag="xt")
                nc.sync.dma_start(
                    out=xt[:].rearrange("p (a d) -> p a d", a=cfg.kin),
                    in_=xdT[:, g * ILV * P:(g + 1) * ILV * P]
                    .rearrange("(a p) d -> p a d", p=P))
                ot = hpool.tile([P, ILV * cfg.nh], bf16, tag="p1o")
                for m in range(ILV):
                    ps = pspool.tile([P, cfg.nh], f32, tag="ps_h",
                                     bufs=4)
                    for kk in range(cfg.kin):
                        nc.tensor.matmul(
                            out=ps[:],
                            lhsT=xt[:, (kk * ILV + m) * P:
                                    (kk * ILV + m + 1) * P],
                            rhs=w1t[:, kk * cfg.nh:(kk + 1) * cfg.nh],
                            start=(kk == 0), stop=(kk == cfg.kin - 1))
                    dst_sl = ot[:, m * cfg.nh:(m + 1) * cfg.nh]
                    if m % 2 == 0:
                        nc.scalar.copy(out=dst_sl, in_=ps[:])
                    else:
                        nc.vector.tensor_copy(out=dst_sl, in_=ps[:])
                nc.scalar.dma_start(
                    out=htab[g * ILV * P:(g + 1) * ILV * P, :]
                    .rearrange("(q i) f -> q (i f)", i=ILV),
                    in_=ot[:])

            # helper: grouped gather of 256B rows for blocks of group gi
            Kgmax = max(int(sum(Ksum[b] for b in grp)) for grp in cfg.groups)

            def gather_group(gi, table):
                # stripe each half's chunks across the 4 SWDGE queues so
                # their DMA rings drain in parallel
                grp = cfg.groups[gi]
                msg = mpool.tile([P, Kgmax * cfg.nh], bf16, tag="msg")
                j0 = 0
                for h in range(2):
                    off = gmeta[2 * gi + h][2]
                    Kh = KA if h == 0 else KB
                    nsub = int(sum(Kh[b] for b in grp))
                    # 4 even stripes on queues 0-3 (period 4 keeps the
                    # 8-lane DMASW sem rotation queue-consistent); each
                    # ring drains at ~8-9ns/descriptor, in parallel.
                    splits = np.linspace(0, nsub, 5).astype(int)
                    for q, qn in enumerate((0, 1, 2, 3)):
                        ks, ke = int(splits[q]), int(splits[q + 1])
                        nq = max(ke - ks, 0)
                        if nq == 0:
                            ks, ke, nq = 0, 1, 1  # keep sem pattern aligned
                        nc.gpsimd.dma_gather(
                            out_ap=msg[:, (j0 + ks) * cfg.nh:
                                       (j0 + ke) * cfg.nh]
                            .rearrange("p (k f) -> p k f", k=nq),
                            in_ap=table[h * cfg.half:(h + 1) * cfg.half, :],
                            idxs_ap=git[:, off + ks * 8:off + ke * 8],
                            num_idxs=nq * P,
                            num_idxs_reg=nq * P,
                            elem_size=cfg.nh,
                            single_packet=False,
                            queue_num=qn)
                    j0 += nsub
                # column offset of block b's half-h chunks inside msg
                moff = {}
                pos = 0
                for h in range(2):
                    Kh = KA if h == 0 else KB
                    for b in grp:
                        moff[(b, h)] = pos
                        pos += int(Kh[b])
                return msg, moff

            # staging tile for the post-AllGather expansion; zeroed once so
            # pad lanes of the phase-3 table are 0.0 rather than garbage
            qa_e = next(q for q in (28, 16, 14, 8, 7, 4, 2, 1)
                        if (cfg.tabn // P) % q == 0)
            zpad = cpool.tile([P, 2 * qa_e * cfg.nh], bf16, tag="zpad")
            nc.scalar.memzero(zpad[:])

            # ---------------- phase 2: zr table (transposed aggregation)
            zrall = hpool.tile([P, cfg.nblk * cfg.nc_out], bf16, tag="zrall")
            for gi in range(len(cfg.groups)):
                msg, moff = gather_group(gi, htab)
                for b in cfg.groups[gi]:
                    K_b = int(Ksum[b])
                    sel = spool.tile([P, K_b * P], bf16, tag="sel")
                    nc.vector.tensor_tensor(
                        out=sel[:].rearrange("p (k f) -> p k f", k=K_b),
                        in0=ldt[:, coff[b]:coff[b + 1], None]
                        .to_broadcast([P, K_b, P]),
                        in1=iot[:, None, :].to_broadcast([P, K_b, P]),
                        op=mybir.AluOpType.is_equal)
                    ps = pspool.tile([P, cfg.nh], f32, tag="ps_agg")
                    nch = 0
                    for h in range(2):
                        Kh = int((KA if h == 0 else KB)[b])
                        m0 = moff[(b, h)]
                        c0 = 0 if h == 0 else int(KA[b])
                        for k in range(Kh):
                            if not zero_bias:
                                nc.vector.tensor_scalar_mul(
                                    out=msg[:, (m0 + k) * cfg.nh:
                                            (m0 + k + 1) * cfg.nh],
                                    in0=msg[:, (m0 + k) * cfg.nh:
                                            (m0 + k + 1) * cfg.nh],
                                    scalar1=ws2t[:, coff[b] + c0 + k:
                                                 coff[b] + c0 + k + 1])
                            nc.tensor.matmul(
                                out=ps[:],
                                lhsT=msg[:, (m0 + k) * cfg.nh:
                                         (m0 + k + 1) * cfg.nh],
                                rhs=sel[:, (c0 + k) * P:(c0 + k + 1) * P],
                                start=(nch == 0), stop=(nch == K_b - 1))
                            nch += 1
                    rt = hpool.tile([P, cfg.nh], bf16, tag="rt")
                    nc.scalar.activation(
                        out=rt[:], in_=ps[:], func=relu,
                        bias=0.0 if zero_bias else b1t[:, 0:1])
                    ps2 = pspool.tile([P, cfg.nc_out], f32, tag="ps_sm")
                    nc.tensor.matmul(out=ps2[:], lhsT=rt[:], rhs=w2t[:],
                                     start=True, stop=True)
                    nc.scalar.mul(
                        out=zrall[:, b * cfg.nc_out:(b + 1) * cfg.nc_out],
                        in_=ps2[:], mul=dvst[:, b:b + 1])
            nc.sync.dma_start(
                out=zrsh[:].rearrange("(b p) c -> p b c", p=P),
                in_=zrall[:].rearrange("p (b c) -> p b c", b=cfg.nblk))

            nc.gpsimd.collective_compute(
                "AllGather", mybir.AluOpType.bypass,
                replica_groups=[list(range(cfg.cores))],
                ins=[zrsh.opt()], outs=[zrtab.opt()])

            # expand compact zr rows into 256B-strided gatherable rows.
            # A direct strided DRAM->DRAM copy costs ~178us (8B descriptors);
            # instead stage through a pre-zeroed SBUF tile so both DMAs use
            # large per-partition-contiguous descriptors.
            qa = qa_e
            Q = qa * P
            NEXP = cfg.tabn // Q
            for c in range(NEXP):
                zin = hpool.tile([P, qa * cfg.nc_out], bf16, tag="zin")
                nc.scalar.dma_start(
                    out=zin[:].rearrange("p (a c) -> p a c", a=qa),
                    in_=zrtab[c * Q:(c + 1) * Q, :]
                    .rearrange("(p a) c -> p a c", p=P))
                zp = zpad[:, (c % 2) * qa * cfg.nh:
                          (c % 2 + 1) * qa * cfg.nh]
                nc.vector.tensor_copy(
                    out=zp.rearrange("p (a f) -> p a f", a=qa)
                    [:, :, 0:cfg.nc_out],
                    in_=zin[:].rearrange("p (a c) -> p a c", a=qa))
                nc.scalar.dma_start(
                    out=htab[c * Q:(c + 1) * Q, :]
                    .rearrange("(p a) f -> p (a f)", p=P),
                    in_=zp)

            # ---------------- phase 3: out.T = sum sel-routed ws3*zr[src]
            oall = hpool.tile([cfg.nc_out, cfg.nblk * P], f32, tag="oall")
            for gi in range(len(cfg.groups)):
                msg, moff = gather_group(gi, htab)
                for b in cfg.groups[gi]:
                    K_b = int(Ksum[b])
                    sel = spool.tile([P, K_b * P], bf16, tag="sel")
                    nc.vector.tensor_tensor(
                        out=sel[:].rearrange("p (k f) -> p k f", k=K_b),
                        in0=ldt[:, coff[b]:coff[b + 1], None]
                        .to_broadcast([P, K_b, P]),
                        in1=iot[:, None, :].to_broadcast([P, K_b, P]),
                        op=mybir.AluOpType.is_equal)
                    ps = pspool.tile([cfg.nc_out, P], f32, tag="ps_sm")
                    nch = 0
                    for h in range(2):
                        Kh = int((KA if h == 0 else KB)[b])
                        m0 = moff[(b, h)]
                        c0 = 0 if h == 0 else int(KA[b])
                        for k in range(Kh):
                            nc.tensor.matmul(
                                out=ps[:],
                                lhsT=msg[:, (m0 + k) * P:
                                         (m0 + k) * P + cfg.nc_out],
                                rhs=sel[:, (c0 + k) * P:(c0 + k + 1) * P],
                                start=(nch == 0), stop=(nch == K_b - 1))
                            nch += 1
                    nc.scalar.copy(
                        out=oall[:, b * P:(b + 1) * P], in_=ps[:])
            nc.vector.tensor_tensor(out=oall[:], in0=oall[:], in1=dvot[:],
                                    op=mybir.AluOpType.mult)
            if not zero_bias:
                nc.vector.tensor_scalar_add(out=oall[:], in0=oall[:],
                                            scalar1=b2t[:, 0:1])
            nc.sync.dma_start(out=out[:, :], in_=oall[:])

    nc.compile()
    return nc


# ------------------------------------------------------------------ driver
def kernel(x, edge_index, W1, b1, W2, b2):
    cfg = FULL
    assert x.shape == (cfg.n, cfg.nin)
    in_maps, KH, zero_bias, ppos = host_prep(
        cfg, np.asarray(x), np.asarray(edge_index), np.asarray(W1),
        np.asarray(b1), np.asarray(W2), np.asarray(b2))
    nc = build_nc(cfg, KH, zero_bias)
    res = run_bass_kernel_spmd(nc, in_maps, core_ids=list(range(cfg.cores)))
    out = np.empty((cfg.n, cfg.nc_out), np.float32)
    for c in range(cfg.cores):
        nodes = np.arange(c * cfg.shard, (c + 1) * cfg.shard)
        lpos = ppos[nodes] - c * cfg.pshard
        out[nodes] = res.results[c]["out"][:, lpos].T
    return out


# revision 27
# speedup vs baseline: 1.1112x; 1.0869x over previous
"""Two-layer GCN (ClinicalGCN) on 8 Trainium2 NeuronCores.

Math (fold dinv[src] into x on host; defer dinv[dst] of layer 1 through
the relu using relu(a*x) = a*relu(x) for a>0, which holds since
self-loops make deg >= 1):
    h_hat[u]  = ((x[u]*dinv[u]) @ W1)            -> htab (full, per core)
    A1[v]     = sum_{e: dst=v} h_hat[src_e]      (segment sum, transposed)
    zero b1:  zr[v] = relu(A1[v]) @ W2           ws3_e = dinv^2[src]*dinv[dst]
    else:     zr[v] = relu(dinv[v]*A1[v]+b1)@W2  ws3_e = dinv[src]*dinv[dst]
    out[i]    = sum_{e: dst=i} ws3_e * zr[src_e] + b2

Device mapping:
  - Phase 1 is computed redundantly on every core (full x streamed in,
    full h_hat table written to the core's own DRAM).  This replaces a
    12.8MB AllGather (~250us) with ~110us of local DMA+PE.
  - Phase 2 dst-shards nodes (49 blocks of 128 per core).  Messages are
    fetched with grouped gpsimd.dma_gather (256B rows, int16 indices,
    table split in two halves), routed into PSUM with per-chunk
    selection matmuls (lhsT=msg, rhs=sel) that directly produce the
    TRANSPOSED aggregate [feat, node] - so relu+b1 is a partition-axis
    ACT op and @W2 needs no transpose.
  - Only the tiny zr table [tabn, 4] bf16 is AllGather'd, then expanded
    into 256B-strided rows so phase 3 can reuse the SAME gather indices
    and selection data as phase 2.
  - Output is accumulated transposed [4, pshard] and fixed up on host.
"""

import math

import ml_dtypes
import numpy as np

import concourse.bacc as bacc
import concourse.bass as bass
import concourse.mybir as mybir
import concourse.tile as tile
from concourse.bass_utils import run_bass_kernel_spmd

P = 128
N_CORES = 8
BF16 = ml_dtypes.bfloat16
ILV = 4        # phase-1 row interleave (1KB DMA descriptors)
GSZ = 4        # dst blocks per gather group


class Cfg:
    def __init__(self, n_nodes, n_in, n_hid, n_out, n_cores=N_CORES):
        assert n_nodes % n_cores == 0
        self.n = n_nodes
        self.nin = n_in
        self.nh = n_hid
        self.nc_out = n_out
        self.cores = n_cores
        self.shard = n_nodes // n_cores           # real nodes per core
        self.nblk = (self.shard + P - 1) // P     # dst blocks per core
        self.pshard = self.nblk * P               # padded nodes per core
        self.tabn = self.pshard * n_cores         # table rows (global)
        assert self.tabn % 2 == 0 and self.tabn % (ILV * P) == 0
        self.half = self.tabn // 2                # rows per table half
        assert self.half <= 32768, "int16 dma_gather index limit"
        self.kin = n_in // P                      # k chunks for x @ W1
        self.ngrp1 = self.tabn // (ILV * P)       # phase-1 write groups
        # phase-2/3 gather groups over the local dst blocks
        self.groups = [list(range(g, min(g + GSZ, self.nblk)))
                       for g in range(0, self.nblk, GSZ)]


FULL = Cfg(50000, 256, 128, 4)


# ---------------------------------------------------------------- host prep
def _wrap_idx(idx):
    """int16 idx array [n*128] -> dma_gather layout [128, n*8]."""
    n = idx.shape[0] // P
    w = idx.reshape(n * 8, 16).T            # [16, n*8]
    return np.tile(w, (8, 1))               # [128, n*8]


def host_prep(cfg: Cfg, x, edge_index, W1, b1, W2, b2):
    """Build per-core input arrays. Pure numpy."""
    n = cfg.n
    src = np.concatenate([edge_index[0], np.arange(n, dtype=np.int64)])
    dst = np.concatenate([edge_index[1], np.arange(n, dtype=np.int64)])
    deg = np.bincount(dst, minlength=n).astype(np.float32)
    dinv = np.where(deg > 0, 1.0 / np.sqrt(deg), 0.0).astype(np.float32)
    zero_bias = not (np.any(b1) or np.any(b2))

    # original padded-table row; only used for the half split, which is
    # invariant under the within-core balancing permutation below
    trow0 = ((src // cfg.shard) * cfg.pshard
             + src % cfg.shard).astype(np.int64)
    half_bit = (trow0 >= cfg.half).astype(np.int64)

    # --- balance dst nodes over blocks (2D LPT on per-half degree) so the
    # per-(block, half) chunk counts quantize to the minimum K everywhere.
    degA = np.zeros(n, np.int64)
    degB = np.zeros(n, np.int64)
    np.add.at(degA, dst[half_bit == 0], 1)
    np.add.at(degB, dst[half_bit == 1], 1)
    ppos = np.zeros(n, np.int64)        # global node -> padded position
    for c in range(cfg.cores):
        nodes = np.arange(c * cfg.shard, (c + 1) * cfg.shard)
        dA = degA[nodes]
        dB = degB[nodes]
        order = np.argsort(-(dA + dB), kind="stable")
        sumA = np.zeros(cfg.nblk, np.int64)
        sumB = np.zeros(cfg.nblk, np.int64)
        used = np.zeros(cfg.nblk, np.int64)
        BIG = 1 << 50
        for v in order:
            score = np.maximum(sumA + dA[v], sumB + dB[v]) + used * 2
            score[used >= P] = BIG
            b = int(score.argmin())
            ppos[nodes[v]] = c * cfg.pshard + b * P + used[b]
            sumA[b] += dA[v]
            sumB[b] += dB[v]
            used[b] += 1

    # phase-1 lhsT: x*dinv padded+transposed; the table row of node v is
    # its balanced position ppos[v] (so phases 2 and 3 share one layout),
    # and columns are further permuted so matmul m of write-group g
    # produces table rows {ILV*P*g + ILV*q + m}.
    trow = ppos[src]
    xd = (x * dinv[:, None]).astype(np.float32)
    X0 = np.zeros((cfg.nin, cfg.tabn), dtype=BF16)
    X0[:, ppos[np.arange(n)]] = xd.T.astype(BF16)
    xdT = (X0.reshape(cfg.nin, cfg.ngrp1, P, ILV)
           .transpose(0, 1, 3, 2).reshape(cfg.nin, cfg.tabn))

    # order edges by destination, group by (core, block, half)
    order = np.argsort(dst, kind="stable")
    dst_s = dst[order]
    trow_s = trow[order]
    src_s = src[order]
    pp_s = ppos[dst_s]
    lslot_s = (pp_s % P).astype(np.float32)
    half_s = (trow_s >= cfg.half).astype(np.int64)
    blk_s = pp_s // P
    ws2_s = dinv[dst_s]

    nblk_total = cfg.cores * cfg.nblk
    cnt = np.zeros((nblk_total, 2), dtype=np.int64)
    np.add.at(cnt, (blk_s, half_s), 1)
    cnt3 = cnt.reshape(cfg.cores, cfg.nblk, 2)
    # chunk counts per (block, half): max across cores (shared SPMD program)
    KH = [np.maximum(1, np.ceil(cnt3[:, :, h].max(axis=0) / P)).astype(int)
          for h in range(2)]

    key = blk_s * 2 + half_s
    order2 = np.argsort(key, kind="stable")
    trow2 = trow_s[order2]
    lslot2 = lslot_s[order2]
    ws2_2 = ws2_s[order2]
    key2 = key[order2]
    starts = np.searchsorted(key2, np.arange(nblk_total * 2 + 1))

    KA, KB = KH
    Ksum = KA + KB
    Ktot = int(Ksum.sum())
    # column offset of each block's chunks in the resident ldst/ws tiles
    coff = np.concatenate([[0], np.cumsum(Ksum)]).astype(int)
    # per-node scale folded into the zr table (dinv^2 when the relu trick
    # absorbs both dinv factors, else dinv), and the final dinv[dst] scale
    dtab = dinv * dinv if zero_bias else dinv
    per_core = []
    for c in range(cfg.cores):
        ldst = np.full((P, Ktot), -1.0, dtype=BF16)
        ws2 = np.zeros((P, Ktot), dtype=np.float32)
        gidx_core = []
        for grp in cfg.groups:
            for h in range(2):
                Kh = KA if h == 0 else KB
                parts = []
                for b in grp:
                    g = (c * cfg.nblk + b) * 2 + h
                    lo, hi = starts[g], starts[g + 1]
                    cnt_e = hi - lo
                    idx = np.zeros(Kh[b] * P, dtype=np.int16)
                    idx[:cnt_e] = trow2[lo:hi] - h * cfg.half
                    parts.append(_wrap_idx(idx))
                    t = np.arange(cnt_e)
                    j0 = coff[b] + (0 if h == 0 else KA[b])
                    ldst[t % P, j0 + t // P] = lslot2[lo:hi].astype(BF16)
                    ws2[t % P, j0 + t // P] = ws2_2[lo:hi]
                gidx_core.append(np.concatenate(parts, axis=1))
        gidx = np.concatenate(gidx_core, axis=1)  # [128, Ktot*8]
        nodes = np.arange(c * cfg.shard, (c + 1) * cfg.shard)
        lpos = ppos[nodes] - c * cfg.pshard
        dvsf = np.zeros(cfg.pshard, np.float32)
        dvsf[lpos] = dtab[nodes]
        dvs = dvsf.reshape(cfg.nblk, P).T.copy()
        dvof = np.zeros(cfg.pshard, np.float32)
        dvof[lpos] = dinv[nodes]
        dvo = np.broadcast_to(dvof.astype(BF16),
                              (cfg.nc_out, cfg.pshard)).copy()
        pc = {"ldst": ldst, "gidx": gidx, "dvs": dvs, "dvo": dvo}
        if not zero_bias:
            pc["ws2"] = ws2
        per_core.append(pc)

    iota = np.broadcast_to(np.arange(P, dtype=np.float32).astype(BF16),
                           (P, P)).copy()
    shared = {
        "xdT": xdT,
        "W1": W1.astype(BF16),
        "W2": W2.astype(BF16),
        "iota": iota,
    }
    if not zero_bias:
        shared["b1c"] = b1.astype(np.float32).reshape(cfg.nh, 1).copy()
        shared["b2c"] = b2.astype(np.float32).reshape(cfg.nc_out, 1).copy()
    in_maps = [{**shared, **pc} for pc in per_core]
    return in_maps, KH, zero_bias, ppos


# --------------------------------------------------------------- bass build
def build_nc(cfg: Cfg, KH, zero_bias):
    f32 = mybir.dt.float32
    bf16 = mybir.dt.bfloat16
    i16 = mybir.dt.int16
    KA, KB = [np.asarray(k, dtype=np.int64) for k in KH]
    Ksum = KA + KB
    Ktot = int(Ksum.sum())
    coff = np.concatenate([[0], np.cumsum(Ksum)]).astype(int)
    # idx column offsets per (grp, half) in the resident gidx tile
    gio = [0]
    gmeta = []   # per (grp, half): (blocks, Ksub, idx_off)
    for grp in cfg.groups:
        for h in range(2):
            Kh = KA if h == 0 else KB
            nidx = int(sum(Kh[b] for b in grp)) * P
            gmeta.append((grp, h, gio[-1]))
            gio.append(gio[-1] + nidx // 16)

    nc = bacc.Bacc("TRN2", target_bir_lowering=False, debug=False,
                   num_devices=cfg.cores, num_swdge_queues=4)

    xdT = nc.dram_tensor("xdT", [cfg.nin, cfg.tabn], bf16,
                         kind="ExternalInput")
    W1 = nc.dram_tensor("W1", [cfg.nin, cfg.nh], bf16, kind="ExternalInput")
    W2 = nc.dram_tensor("W2", [cfg.nh, cfg.nc_out], bf16,
                        kind="ExternalInput")
    iota = nc.dram_tensor("iota", [P, P], bf16, kind="ExternalInput")
    gidx = nc.dram_tensor("gidx", [P, gio[-1]], i16, kind="ExternalInput")
    ldst = nc.dram_tensor("ldst", [P, Ktot], bf16, kind="ExternalInput")
    dvs = nc.dram_tensor("dvs", [P, cfg.nblk], f32, kind="ExternalInput")
    dvo = nc.dram_tensor("dvo", [cfg.nc_out, cfg.pshard], bf16,
                         kind="ExternalInput")
    if not zero_bias:
        ws2 = nc.dram_tensor("ws2", [P, Ktot], f32, kind="ExternalInput")
        b1c = nc.dram_tensor("b1c", [cfg.nh, 1], f32, kind="ExternalInput")
        b2c = nc.dram_tensor("b2c", [cfg.nc_out, 1], f32,
                             kind="ExternalInput")
    out = nc.dram_tensor("out", [cfg.nc_out, cfg.pshard], f32,
                         kind="ExternalOutput")

    relu = mybir.ActivationFunctionType.Relu

    with tile.TileContext(nc) as tc:
        with (
            tc.tile_pool(name="const", bufs=1) as cpool,
            tc.tile_pool(name="x", bufs=3) as xpool,
            tc.tile_pool(name="h", bufs=3) as hpool,
            tc.tile_pool(name="msg", bufs=3) as mpool,
            tc.tile_pool(name="sel", bufs=2) as spool,
            tc.tile_pool(name="ps", bufs=2, space="PSUM") as pspool,
            tc.tile_pool(name="dram", bufs=1, space="DRAM") as dram,
        ):
            # ---- resident constants / metadata
            w1t = cpool.tile([P, cfg.kin * cfg.nh], bf16, tag="w1")
            nc.sync.dma_start(
                out=w1t[:].rearrange("p (a d) -> p a d", a=cfg.kin),
                in_=W1[:].rearrange("(a p) d -> p a d", p=P))
            w2t = cpool.tile([cfg.nh, cfg.nc_out], bf16, tag="w2")
            nc.sync.dma_start(out=w2t[:], in_=W2[:])
            iot = cpool.tile([P, P], bf16, tag="iota")
            nc.sync.dma_start(out=iot[:], in_=iota[:])
            git = cpool.tile([P, gio[-1]], i16, tag="gidx")
            nc.sync.dma_start(out=git[:], in_=gidx[:])
            ldt = cpool.tile([P, Ktot], bf16, tag="ldst")
            nc.sync.dma_start(out=ldt[:], in_=ldst[:])
            dvst = cpool.tile([P, cfg.nblk], f32, tag="dvs")
            nc.sync.dma_start(out=dvst[:], in_=dvs[:])
            dvot = cpool.tile([cfg.nc_out, cfg.pshard], bf16, tag="dvo")
            nc.sync.dma_start(out=dvot[:], in_=dvo[:])
            if not zero_bias:
                ws2t = cpool.tile([P, Ktot], f32, tag="ws2")
                nc.sync.dma_start(out=ws2t[:], in_=ws2[:])
                b1t = cpool.tile([cfg.nh, 1], f32, tag="b1")
                nc.sync.dma_start(out=b1t[:], in_=b1c[:])
                b2t = cpool.tile([cfg.nc_out, 1], f32, tag="b2")
                nc.sync.dma_start(out=b2t[:], in_=b2c[:])

            htab = dram.tile([cfg.tabn, cfg.nh], bf16)
            zrsh = dram.tile([cfg.pshard, cfg.nc_out], bf16)
            zrtab = dram.tile([cfg.tabn, cfg.nc_out], bf16,
                              addr_space="Shared")

            # ---------------- phase 1: full h_hat table, written interleaved
            for g in range(cfg.ngrp1):
                xt = xpool.tile([P, cfg.kin * ILV * P], bf16, tag="xt")
                nc.sync.dma_start(
                    out=xt[:].rearrange("p (a d) -> p a d", a=cfg.kin),
                    in_=xdT[:, g * ILV * P:(g + 1) * ILV * P]
                    .rearrange("(a p) d -> p a d", p=P))
                ot = hpool.tile([P, ILV * cfg.nh], bf16, tag="p1o")
                for m in range(ILV):
                    ps = pspool.tile([P, cfg.nh], f32, tag="ps_h",
                                     bufs=4)
                    for kk in range(cfg.kin):
                        nc.tensor.matmul(
                            out=ps[:],
                            lhsT=xt[:, (kk * ILV + m) * P:
                                    (kk * ILV + m + 1) * P],
                            rhs=w1t[:, kk * cfg.nh:(kk + 1) * cfg.nh],
                            start=(kk == 0), stop=(kk == cfg.kin - 1))
                    dst_sl = ot[:, m * cfg.nh:(m + 1) * cfg.nh]
                    if m % 2 == 0:
                        nc.scalar.copy(out=dst_sl, in_=ps[:])
                    else:
                        nc.vector.tensor_copy(out=dst_sl, in_=ps[:])
                nc.scalar.dma_start(
                    out=htab[g * ILV * P:(g + 1) * ILV * P, :]
                    .rearrange("(q i) f -> q (i f)", i=ILV),
                    in_=ot[:])

            # helper: grouped gather of 256B rows for blocks of group gi
            Kgmax = max(int(sum(Ksum[b] for b in grp)) for grp in cfg.groups)

            def raw_gather(out_ap, in_ap, idxs_ap, num_idxs, elem_size,
                           elem_step, queue_num):
                """bass.dma_gather minus the elem_size%256 assert: the ISA
                stride field is in 256B units (stride must be %256) but the
                copied elem_size per descriptor can be smaller."""
                ng = nc.gpsimd
                stride_bytes = elem_step * 2  # bf16 table
                assert stride_bytes % 256 == 0
                _in_ap = ng.lower_ap_dma(in_ap, for_custom_bir_dma=True)
                _idxs_ap = ng.lower_ap(idxs_ap)
                _out_ap = ng.lower_ap(out_ap)
                return ng.add_instruction(
                    mybir.InstDMAGatherAnt(
                        name=nc.get_next_instruction_name(),
                        ins=[*_in_ap, _idxs_ap,
                             ng.lower_val_access(ng.to_reg(num_idxs))],
                        outs=[_out_ap],
                        transpose=False,
                        num_idxs=num_idxs,
                        elem_size=elem_size,
                        stride_bytes_256=stride_bytes // 256,
                        gen_mode=0,
                        single_packet=False,
                        queue_num=queue_num,
                        sbuf_tokens_per_rank=0,
                        sbuf_free_dim_per_rank=0,
                        sbuf_free_dim_pad_per_rank=0,
                        sbuf_byte_offset=0))

            def gather_group(gi, table, ew=None):
                # stripe each half's chunks across the 4 SWDGE queues so
                # their DMA rings drain in parallel
                grp = cfg.groups[gi]
                full = ew is None
                ew = cfg.nh if full else ew
                msg = mpool.tile([P, Kgmax * ew], bf16,
                                 tag="msg" if full else "msg3")
                j0 = 0
                for h in range(2):
                    off = gmeta[2 * gi + h][2]
                    Kh = KA if h == 0 else KB
                    nsub = int(sum(Kh[b] for b in grp))
                    # 4 even stripes on queues 0-3 (period 4 keeps the
                    # 8-lane DMASW sem rotation queue-consistent); each
                    # ring drains at ~8-9ns/descriptor, in parallel.
                    splits = np.linspace(0, nsub, 5).astype(int)
                    for q, qn in enumerate((0, 1, 2, 3)):
                        ks, ke = int(splits[q]), int(splits[q + 1])
                        nq = max(ke - ks, 0)
                        if nq == 0:
                            ks, ke, nq = 0, 1, 1  # keep sem pattern aligned
                        oap = (msg[:, (j0 + ks) * ew:(j0 + ke) * ew]
                               .rearrange("p (k f) -> p k f", k=nq))
                        iap = git[:, off + ks * 8:off + ke * 8]
                        if full:
                            nc.gpsimd.dma_gather(
                                out_ap=oap,
                                in_ap=table[h * cfg.half:
                                            (h + 1) * cfg.half, :],
                                idxs_ap=iap,
                                num_idxs=nq * P,
                                num_idxs_reg=nq * P,
                                elem_size=cfg.nh,
                                single_packet=False,
                                queue_num=qn)
                        else:
                            raw_gather(
                                oap,
                                table[h * cfg.half:(h + 1) * cfg.half, 0:ew],
                                iap, nq * P, ew, cfg.nh, qn)
                    j0 += nsub
                # column offset of block b's half-h chunks inside msg
                moff = {}
                pos = 0
                for h in range(2):
                    Kh = KA if h == 0 else KB
                    for b in grp:
                        moff[(b, h)] = pos
                        pos += int(Kh[b])
                return msg, moff

            # staging tile for the post-AllGather expansion; zeroed once so
            # pad lanes of the phase-3 table are 0.0 rather than garbage
            qa_e = next(q for q in (28, 16, 14, 8, 7, 4, 2, 1)
                        if (cfg.tabn // P) % q == 0)
            zpad = cpool.tile([P, 2 * qa_e * cfg.nh], bf16, tag="zpad")
            nc.scalar.memzero(zpad[:])

            # ---------------- phase 2: zr table (transposed aggregation)
            zrall = hpool.tile([P, cfg.nblk * cfg.nc_out], bf16, tag="zrall")
            for gi in range(len(cfg.groups)):
                msg, moff = gather_group(gi, htab)
                for b in cfg.groups[gi]:
                    K_b = int(Ksum[b])
                    sel = spool.tile([P, K_b * P], bf16, tag="sel")
                    nc.vector.tensor_tensor(
                        out=sel[:].rearrange("p (k f) -> p k f", k=K_b),
                        in0=ldt[:, coff[b]:coff[b + 1], None]
                        .to_broadcast([P, K_b, P]),
                        in1=iot[:, None, :].to_broadcast([P, K_b, P]),
                        op=mybir.AluOpType.is_equal)
                    ps = pspool.tile([P, cfg.nh], f32, tag="ps_agg")
                    nch = 0
                    for h in range(2):
                        Kh = int((KA if h == 0 else KB)[b])
                        m0 = moff[(b, h)]
                        c0 = 0 if h == 0 else int(KA[b])
                        for k in range(Kh):
                            if not zero_bias:
                                nc.vector.tensor_scalar_mul(
                                    out=msg[:, (m0 + k) * cfg.nh:
                                            (m0 + k + 1) * cfg.nh],
                                    in0=msg[:, (m0 + k) * cfg.nh:
                                            (m0 + k + 1) * cfg.nh],
                                    scalar1=ws2t[:, coff[b] + c0 + k:
                                                 coff[b] + c0 + k + 1])
                            nc.tensor.matmul(
                                out=ps[:],
                                lhsT=msg[:, (m0 + k) * cfg.nh:
                                         (m0 + k + 1) * cfg.nh],
                                rhs=sel[:, (c0 + k) * P:(c0 + k + 1) * P],
                                start=(nch == 0), stop=(nch == K_b - 1))
                            nch += 1
                    rt = hpool.tile([P, cfg.nh], bf16, tag="rt")
                    nc.scalar.activation(
                        out=rt[:], in_=ps[:], func=relu,
                        bias=0.0 if zero_bias else b1t[:, 0:1])
                    ps2 = pspool.tile([P, cfg.nc_out], f32, tag="ps_sm")
                    nc.tensor.matmul(out=ps2[:], lhsT=rt[:], rhs=w2t[:],
                                     start=True, stop=True)
                    nc.scalar.mul(
                        out=zrall[:, b * cfg.nc_out:(b + 1) * cfg.nc_out],
                        in_=ps2[:], mul=dvst[:, b:b + 1])
            nc.sync.dma_start(
                out=zrsh[:].rearrange("(b p) c -> p b c", p=P),
                in_=zrall[:].rearrange("p (b c) -> p b c", b=cfg.nblk))

            nc.gpsimd.collective_compute(
                "AllGather", mybir.AluOpType.bypass,
                replica_groups=[list(range(cfg.cores))],
                ins=[zrsh.opt()], outs=[zrtab.opt()])

            # expand compact zr rows into 256B-strided gatherable rows.
            # A direct strided DRAM->DRAM copy costs ~178us (8B descriptors);
            # instead stage through a pre-zeroed SBUF tile so both DMAs use
            # large per-partition-contiguous descriptors.
            qa = qa_e
            Q = qa * P
            NEXP = cfg.tabn // Q
            for c in range(NEXP):
                zin = hpool.tile([P, qa * cfg.nc_out], bf16, tag="zin")
                nc.scalar.dma_start(
                    out=zin[:].rearrange("p (a c) -> p a c", a=qa),
                    in_=zrtab[c * Q:(c + 1) * Q, :]
                    .rearrange("(p a) c -> p a c", p=P))
                zp = zpad[:, (c % 2) * qa * cfg.nh:
                          (c % 2 + 1) * qa * cfg.nh]
                nc.vector.tensor_copy(
                    out=zp.rearrange("p (a f) -> p a f", a=qa)
                    [:, :, 0:cfg.nc_out],
                    in_=zin[:].rearrange("p (a c) -> p a c", a=qa))
                nc.scalar.dma_start(
                    out=htab[c * Q:(c + 1) * Q, :]
                    .rearrange("(p a) f -> p (a f)", p=P),
                    in_=zp)

            # ---------------- phase 3: out.T = sum sel-routed ws3*zr[src]
            EW3 = 8     # bf16 lanes gathered per edge in phase 3 (16B)
            oall = hpool.tile([cfg.nc_out, cfg.nblk * P], f32, tag="oall")
            for gi in range(len(cfg.groups)):
                msg, moff = gather_group(gi, htab, ew=EW3)
                for b in cfg.groups[gi]:
                    K_b = int(Ksum[b])
                    sel = spool.tile([P, K_b * P], bf16, tag="sel")
                    nc.vector.tensor_tensor(
                        out=sel[:].rearrange("p (k f) -> p k f", k=K_b),
                        in0=ldt[:, coff[b]:coff[b + 1], None]
                        .to_broadcast([P, K_b, P]),
                        in1=iot[:, None, :].to_broadcast([P, K_b, P]),
                        op=mybir.AluOpType.is_equal)
                    ps = pspool.tile([cfg.nc_out, P], f32, tag="ps_sm")
                    nch = 0
                    for h in range(2):
                        Kh = int((KA if h == 0 else KB)[b])
                        m0 = moff[(b, h)]
                        c0 = 0 if h == 0 else int(KA[b])
                        for k in range(Kh):
                            nc.tensor.matmul(
                                out=ps[:],
                                lhsT=msg[:, (m0 + k) * EW3:
                                         (m0 + k) * EW3 + cfg.nc_out],
                                rhs=sel[:, (c0 + k) * P:(c0 + k + 1) * P],
                                start=(nch == 0), stop=(nch == K_b - 1))
                            nch += 1
                    nc.scalar.copy(
                        out=oall[:, b * P:(b + 1) * P], in_=ps[:])
            nc.vector.tensor_tensor(out=oall[:], in0=oall[:], in1=dvot[:],
                                    op=mybir.AluOpType.mult)
            if not zero_bias:
                nc.vector.tensor_scalar_add(out=oall[:], in0=oall[:],
                                            scalar1=b2t[:, 0:1])
            nc.sync.dma_start(out=out[:, :], in_=oall[:])

    nc.compile()
    return nc


# ------------------------------------------------------------------ driver
def kernel(x, edge_index, W1, b1, W2, b2):
    cfg = FULL
    assert x.shape == (cfg.n, cfg.nin)
    in_maps, KH, zero_bias, ppos = host_prep(
        cfg, np.asarray(x), np.asarray(edge_index), np.asarray(W1),
        np.asarray(b1), np.asarray(W2), np.asarray(b2))
    nc = build_nc(cfg, KH, zero_bias)
    res = run_bass_kernel_spmd(nc, in_maps, core_ids=list(range(cfg.cores)))
    out = np.empty((cfg.n, cfg.nc_out), np.float32)
    for c in range(cfg.cores):
        nodes = np.arange(c * cfg.shard, (c + 1) * cfg.shard)
        lpos = ppos[nodes] - c * cfg.pshard
        out[nodes] = res.results[c]["out"][:, lpos].T
    return out
